# revision 5
# baseline (speedup 1.0000x reference)
"""Trainium2 Bass kernel for nn_MaxPoolingMatching.

Reference computation (per batch b):
    v1[l,p,:] = l2norm(s1[l,:] * k[p,:])        # over d
    v2[m,p,:] = l2norm(s2[m,:] * k[p,:])
    cos[l,m,p] = <v1[l,p,:], v2[m,p,:]>
    out[l,p]   = max_m cos[l,m,p]

Rewritten as
    Y[l,m,p]  = sum_d s1[l,d] * k2[p,d] * s2[m,d]        (k2 = k*k)
    out[l,p]  = rinv1[l,p] * max_m ( Y[l,m,p] * rinv2[m,p] )
where rinv{1,2} are inverse weighted norms; the positive rinv1 factor
commutes with the max.

Mapping to the NeuronCore (batch-parallel across 8 cores, 2 batches/core):
  - main matmuls: stationary = s1T chunk (fp16, reused across all P
    perspectives -> cheap FWL weight loads), moving = rhsK = k2-scaled s2T
    (fp16), N=512 streams covering 2 perspectives each, one PSUM bank per
    perspective pair
  - rhsK build (40 per-partition-scale ops) split between ScalarE and GPSIMD
  - epilogue: fused multiply(max rinv2)+max-reduce custom DVE op straight
    from PSUM (the span floor), rinv2 broadcast tile in fp16 via DRAM bounce
  - input transposes on PE; squares + norm matmuls feed rinv1/rinv2
"""

import sys

import numpy as np

if "/opt/trn_rl_repo" not in sys.path:
    sys.path.insert(0, "/opt/trn_rl_repo")

B, L, D, P = 16, 256, 256, 20
NCORES = 8
BLOC = B // NCORES  # batches per core
LC = L // 128  # l chunks (partition-dim tiles)
DC = D // 128  # d chunks (contraction tiles)
NPAIR = P // 2  # perspective pairs (one PSUM bank each)

_NC_CACHE = {}


def _register_mulmax_op():
    from concourse import dve_ops
    from concourse.dve_spec import Spec, Src0, Src1, AluOp, lower
    from concourse.dve_spec import _has_src1 as has_src1
    from concourse.dve_ops import DveOpSpec

    for op in dve_ops.OPS:
        if op.name == "MULMAX_ANT":
            return op

    def _ref(in0, in1, c0, c1, c2):
        out = in0 * in1
        return out, out.max(axis=-1, keepdims=True)

    spec = Spec(body=Src0 * Src1, accum=AluOp.MAX, reference=_ref)
    op = dve_ops.DveOp("MULMAX_ANT", spec, subdim=False, uops_sha={})
    dve_ops.OPS.append(op)
    dve_ops._SUB_OPCODE_FOR_NAME[op.name] = (
        dve_ops._CUSTOM_DVE_ROW_BASE + len(dve_ops.OPS) - 1
    )
    dve_ops.CUSTOM_DVE_SPECS[op.name] = spec
    for ver in ("v3", "v4"):
        try:
            s = DveOpSpec(
                name=op.name,
                opcode=dve_ops.get_dve_sub_opcode(op.name),
                uops=lower(spec, ver=ver),
                rd1_en=has_src1(spec),
            )
            op.uops_sha[ver] = s.sha(ver)
        except Exception:
            pass
    return op


def build_nc():
    import concourse.bass as bass
    import concourse.bacc as bacc
    import concourse.tile as tile
    from concourse import mybir
    from concourse.masks import make_identity
    from contextlib import ExitStack

    f32 = mybir.dt.float32
    f16 = mybir.dt.float16
    Act = mybir.ActivationFunctionType

    mulmax_op = _register_mulmax_op()

    nc = bacc.Bacc("TRN2", target_bir_lowering=False, debug=False)
    s1_d = nc.dram_tensor("sent1", [BLOC, L, D], f32, kind="ExternalInput").ap()
    s2_d = nc.dram_tensor("sent2", [BLOC, L, D], f32, kind="ExternalInput").ap()
    kr_d = nc.dram_tensor("kernel", [P, D], f32, kind="ExternalInput").ap()
    out_d = nc.dram_tensor("out", [BLOC, L, P], f32, kind="ExternalOutput").ap()

    with ExitStack() as ctx:
        tc = ctx.enter_context(tile.TileContext(nc))
        consts = ctx.enter_context(tc.tile_pool(name="consts", bufs=1))
        nat = ctx.enter_context(tc.tile_pool(name="nat", bufs=4))
        big = ctx.enter_context(tc.tile_pool(name="big", bufs=1))
        small = ctx.enter_context(tc.tile_pool(name="small", bufs=4))
        outp = ctx.enter_context(tc.tile_pool(name="outp", bufs=4))
        dramp = ctx.enter_context(tc.tile_pool(name="dram", bufs=1, space="DRAM"))
        ps_tr = ctx.enter_context(tc.tile_pool(name="ps_tr", bufs=2, space="PSUM"))
        ps_nrm = ctx.enter_context(tc.tile_pool(name="ps_nrm", bufs=1, space="PSUM"))
        ps_z = ctx.enter_context(tc.tile_pool(name="ps_z", bufs=5, space="PSUM"))

        # ---- constants -------------------------------------------------
        ident = consts.tile([128, 128], f32, tag="ident")
        make_identity(nc, ident)
        eps_t = consts.tile([128, 1], f32, tag="eps")
        nc.vector.memset(eps_t, 1e-12)

        kr = consts.tile([P, D], f32, tag="kr")
        nc.sync.dma_start(out=kr, in_=kr_d)
        k2 = consts.tile([P, D], f32, tag="k2")
        nc.gpsimd.tensor_mul(k2, kr, kr)

        # k2T[d, dc, p]: transpose k2 through the PE
        k2T = consts.tile([128, DC, P], f32, tag="k2T")
        for dcc in range(DC):
            pst = ps_tr.tile([128, 512], f32, tag="tr")
            nc.tensor.transpose(
                pst[:, :P], k2[:, dcc * 128 : (dcc + 1) * 128], ident[:P, :P]
            )
            nc.scalar.copy(out=k2T[:, dcc, :], in_=pst[:, :P])

        # ---- s2 pipeline: load, transpose, square, rinv2 ----------------
        # s2T layout: [128(d), dc, b, m] fp32 (feeds rhsK build)
        s2T = big.tile([128, DC, BLOC, L], f32, tag="s2T")
        s2Tsq = big.tile([128, DC, BLOC, L], f32, tag="s2Tsq")
        for b in range(BLOC):
            natt2 = {}
            for lc in range(LC):
                natt = nat.tile([128, D], f32, tag="nat")
                nc.sync.dma_start(out=natt, in_=s2_d[b, lc * 128 : (lc + 1) * 128, :])
                natt2[lc] = natt
            # psum slab [128, 4*128] laid out (dc, lc) to match s2T free order
            slab = ps_tr.tile([128, 512], f32, tag="tr")
            for dcc in range(DC):
                for lc in range(LC):
                    o = (dcc * 2 + lc) * 128
                    nc.tensor.transpose(
                        slab[:, o : o + 128],
                        natt2[lc][:, dcc * 128 : (dcc + 1) * 128],
                        ident,
                    )
            slab3 = slab.rearrange("a (b c) -> a b c", b=DC)
            nc.scalar.copy(out=s2T[:, :, b, :], in_=slab3)
            nc.scalar.activation(out=s2Tsq[:, :, b, :], in_=slab3, func=Act.Square)

        # rinv2T[p, (b,m)] -> fp16, bounce through DRAM for the broadcast
        n2 = ps_nrm.tile([128, 512], f32, tag="nrm")
        for dcc in range(DC):
            nc.tensor.matmul(
                n2[:P, :],
                k2T[:, dcc, :],
                s2Tsq[:, dcc, :, :].rearrange("a b c -> a (b c)"),
                start=(dcc == 0),
                stop=(dcc == DC - 1),
            )
        sq2 = small.tile([P, BLOC * L], f32, tag="sq2")
        nc.scalar.activation(
            out=sq2, in_=n2[:P, :], func=Act.Sqrt, bias=eps_t[:P], scale=1.0
        )
        rinv2T = small.tile([P, BLOC * L], f16, tag="rinv2T")
        with nc.allow_low_precision(reason="rinv2 broadcast tile in fp16 is ample"):
            nc.vector.reciprocal(out=rinv2T, in_=sq2)
        r2d = dramp.tile([P, BLOC, L], f16, tag="r2d")
        nc.sync.dma_start(out=r2d, in_=rinv2T.rearrange("a (b c) -> a b c", b=BLOC))

        # broadcast to r2b[128, b, p, m] fp16 via partition-step-0 DRAM reads,
        # in chunks of 10 perspectives so the epilogue can start early
        r2b = big.tile([128, BLOC, P, L], f16, tag="r2b")
        r2d_flat = r2d.rearrange("a b c -> (a b c)")
        PCH = 10
        for b in range(BLOC):
            for ps0 in range(0, P, PCH):
                chunk = bass.AP(
                    tensor=r2d_flat.tensor,
                    offset=r2d_flat.offset + ps0 * BLOC * L + b * L,
                    ap=[[0, 128], [BLOC * L, PCH], [1, L]],
                )
                nc.sync.dma_start(out=r2b[:, b, ps0 : ps0 + PCH, :], in_=chunk)

        # ---- s1 pipeline: load, transpose (fp16 + squares), rinv1 -------
        s1T = big.tile([128, DC, BLOC, L], f16, tag="s1T")
        s1Tsq = big.tile([128, DC, BLOC, L], f32, tag="s1Tsq")
        for b in range(BLOC):
            natt1 = {}
            for lc in range(LC):
                natt = nat.tile([128, D], f32, tag="nat")
                nc.sync.dma_start(out=natt, in_=s1_d[b, lc * 128 : (lc + 1) * 128, :])
                natt1[lc] = natt
            slab = ps_tr.tile([128, 512], f32, tag="tr")
            for dcc in range(DC):
                for lc in range(LC):
                    o = (dcc * 2 + lc) * 128
                    nc.tensor.transpose(
                        slab[:, o : o + 128],
                        natt1[lc][:, dcc * 128 : (dcc + 1) * 128],
                        ident,
                    )
            slab3 = slab.rearrange("a (b c) -> a b c", b=DC)
            nc.scalar.copy(out=s1T[:, :, b, :], in_=slab3)
            nc.scalar.activation(out=s1Tsq[:, :, b, :], in_=slab3, func=Act.Square)

        rinv1 = {}
        for b in range(BLOC):
            for lc in range(LC):
                n1 = ps_nrm.tile([128, 512], f32, tag="nrm")
                for dcc in range(DC):
                    nc.tensor.matmul(
                        n1[:, :P],
                        s1Tsq[:, dcc, b, lc * 128 : (lc + 1) * 128],
                        k2T[:, dcc, :],
                        start=(dcc == 0),
                        stop=(dcc == DC - 1),
                    )
                sq1 = small.tile([128, P], f32, tag="sq1")
                nc.scalar.activation(
                    out=sq1, in_=n1[:, :P], func=Act.Sqrt, bias=eps_t, scale=1.0
                )
                r1 = small.tile([128, P], f32, tag="rinv1")
                nc.vector.reciprocal(out=r1, in_=sq1)
                rinv1[b, lc] = r1

        # ---- rhsK build: k2-scaled s2T in fp16 --------------------------
        # rhsK[d, dc, b, p, m]; pairs 0-4 on GPSIMD (ready early), 5-9 on
        # ScalarE (catches up while the main loop runs)
        rhsK = big.tile([128, DC, BLOC, P, L], f16, tag="rhsK")
        for pair in range(NPAIR):
            eng = nc.gpsimd if pair < 5 else nc.scalar
            for dcc in range(DC):
                for p in (2 * pair, 2 * pair + 1):
                    dst = rhsK[:, dcc, :, p, :]
                    src = s2T[:, dcc, :, :]
                    if eng is nc.scalar:
                        nc.scalar.activation(
                            out=dst,
                            in_=src,
                            func=Act.Copy,
                            scale=k2T[:, dcc, p : p + 1],
                        )
                    else:
                        nc.gpsimd.tensor_scalar_mul(dst, src, k2T[:, dcc, p : p + 1])

        # ---- main loop --------------------------------------------------
        for b in range(BLOC):
            for lc in range(LC):
                maxt = outp.tile([128, P], f32, tag="maxt")
                for pair in range(NPAIR):
                    psz = ps_z.tile([128, 512], f32, tag="z")
                    for dcc in range(DC):
                        nc.tensor.matmul(
                            psz,
                            s1T[:, dcc, b, lc * 128 : (lc + 1) * 128],
                            rhsK[:, dcc, b, 2 * pair : 2 * pair + 2, :].rearrange(
                                "a b c -> a (b c)"
                            ),
                            start=(dcc == 0),
                            stop=(dcc == DC - 1),
                        )
                    for i in range(2):
                        p = 2 * pair + i
                        nc.vector._custom_dve(
                            mulmax_op,
                            out=psz[:, i * L : (i + 1) * L],
                            in0=psz[:, i * L : (i + 1) * L],
                            in1=r2b[:, b, p, :],
                            accum_out=maxt[:, p : p + 1],
                        )
                outt = outp.tile([128, P], f32, tag="outt")
                nc.vector.tensor_mul(outt, maxt, rinv1[b, lc])
                nc.sync.dma_start(out=out_d[b, lc * 128 : (lc + 1) * 128, :], in_=outt)

    nc.compile()
    return nc


def _get_nc():
    if "nc" not in _NC_CACHE:
        _NC_CACHE["nc"] = build_nc()
    return _NC_CACHE["nc"]


def run(inputs, trace=False, trace_kwargs=None):
    from concourse.bass_utils import run_bass_kernel_spmd

    nc = _get_nc()
    sent1 = np.ascontiguousarray(np.asarray(inputs["sent1"], dtype=np.float32))
    sent2 = np.ascontiguousarray(np.asarray(inputs["sent2"], dtype=np.float32))
    kr = np.ascontiguousarray(np.asarray(inputs["kernel"], dtype=np.float32))
    in_maps = [
        {
            "sent1": sent1[i * BLOC : (i + 1) * BLOC],
            "sent2": sent2[i * BLOC : (i + 1) * BLOC],
            "kernel": kr,
        }
        for i in range(NCORES)
    ]
    res = run_bass_kernel_spmd(
        nc,
        in_maps,
        core_ids=list(range(NCORES)),
        trace=trace,
        **(trace_kwargs or {}),
    )
    out = np.concatenate([res.results[i]["out"] for i in range(NCORES)], axis=0)
    return out, res


def kernel(sent1, sent2, kernel):
    out, _ = run({"sent1": sent1, "sent2": sent2, "kernel": kernel})
    return out


# revision 8
# speedup vs baseline: 2.8247x; 2.8247x over previous
"""Trainium2 Bass kernel for nn_MaxPoolingMatching.

Reference computation (per batch b):
    v1[l,p,:] = l2norm(s1[l,:] * k[p,:])        # over d
    v2[m,p,:] = l2norm(s2[m,:] * k[p,:])
    cos[l,m,p] = <v1[l,p,:], v2[m,p,:]>
    out[l,p]   = max_m cos[l,m,p]

Rewritten as
    Y[l,m,p]  = sum_d s1[l,d] * k2[p,d] * s2[m,d]        (k2 = k*k)
    out[l,p]  = rinv1[l,p] * max_m ( Y[l,m,p] * rinv2[m,p] )
where rinv{1,2} are inverse weighted norms; the positive rinv1 factor
commutes with the max.

Mapping to the NeuronCore (batch-parallel across 8 cores, 2 batches/core):
  - main matmuls: stationary = s1T chunk (fp16, reused across all P
    perspectives -> cheap FWL weight loads), moving = rhsK = k2-scaled s2T
    (fp16), N=512 streams covering 2 perspectives each, one PSUM bank per
    perspective pair
  - rhsK build (40 per-partition-scale ops) split between ScalarE and GPSIMD
  - epilogue: fused multiply(max rinv2)+max-reduce custom DVE op straight
    from PSUM (the span floor), rinv2 broadcast tile in fp16 via DRAM bounce
  - input transposes on PE; squares + norm matmuls feed rinv1/rinv2
"""

import sys

import numpy as np

if "/opt/trn_rl_repo" not in sys.path:
    sys.path.insert(0, "/opt/trn_rl_repo")

B, L, D, P = 16, 256, 256, 20
NCORES = 8
BLOC = B // NCORES  # batches per core
LC = L // 128  # l chunks (partition-dim tiles)
DC = D // 128  # d chunks (contraction tiles)
NPAIR = P // 2  # perspective pairs (one PSUM bank each)

_NC_CACHE = {}


def _register_mulmax_op():
    from concourse import dve_ops
    from concourse.dve_spec import Spec, Src0, Src1, AluOp, lower
    from concourse.dve_spec import _has_src1 as has_src1
    from concourse.dve_ops import DveOpSpec

    for op in dve_ops.OPS:
        if op.name == "MULMAX_ANT":
            return op

    def _ref(in0, in1, c0, c1, c2):
        out = in0 * in1
        return out, out.max(axis=-1, keepdims=True)

    spec = Spec(body=Src0 * Src1, accum=AluOp.MAX, reference=_ref)
    op = dve_ops.DveOp("MULMAX_ANT", spec, subdim=False, uops_sha={})
    dve_ops.OPS.append(op)
    dve_ops._SUB_OPCODE_FOR_NAME[op.name] = (
        dve_ops._CUSTOM_DVE_ROW_BASE + len(dve_ops.OPS) - 1
    )
    dve_ops.CUSTOM_DVE_SPECS[op.name] = spec
    for ver in ("v3", "v4"):
        try:
            s = DveOpSpec(
                name=op.name,
                opcode=dve_ops.get_dve_sub_opcode(op.name),
                uops=lower(spec, ver=ver),
                rd1_en=has_src1(spec),
            )
            op.uops_sha[ver] = s.sha(ver)
        except Exception:
            pass
    return op


def build_nc():
    import concourse.bass as bass
    import concourse.bacc as bacc
    import concourse.tile as tile
    from concourse import mybir
    from concourse.masks import make_identity
    from contextlib import ExitStack

    f32 = mybir.dt.float32
    f16 = mybir.dt.float16
    Act = mybir.ActivationFunctionType

    mulmax_op = _register_mulmax_op()

    nc = bacc.Bacc("TRN2", target_bir_lowering=False, debug=False)
    s1_d = nc.dram_tensor("sent1", [BLOC, L, D], f32, kind="ExternalInput").ap()
    s2_d = nc.dram_tensor("sent2", [BLOC, L, D], f32, kind="ExternalInput").ap()
    kr_d = nc.dram_tensor("kernel", [P, D], f32, kind="ExternalInput").ap()
    out_d = nc.dram_tensor("out", [BLOC, L, P], f32, kind="ExternalOutput").ap()

    with ExitStack() as ctx:
        tc = ctx.enter_context(tile.TileContext(nc))
        consts = ctx.enter_context(tc.tile_pool(name="consts", bufs=1))
        nat = ctx.enter_context(tc.tile_pool(name="nat", bufs=4))
        big = ctx.enter_context(tc.tile_pool(name="big", bufs=1))
        small = ctx.enter_context(tc.tile_pool(name="small", bufs=4))
        outp = ctx.enter_context(tc.tile_pool(name="outp", bufs=4))
        dramp = ctx.enter_context(tc.tile_pool(name="dram", bufs=1, space="DRAM"))
        ps_tr = ctx.enter_context(tc.tile_pool(name="ps_tr", bufs=2, space="PSUM"))
        ps_nrm = ctx.enter_context(tc.tile_pool(name="ps_nrm", bufs=1, space="PSUM"))
        ps_z = ctx.enter_context(tc.tile_pool(name="ps_z", bufs=5, space="PSUM"))

        # ---- constants -------------------------------------------------
        ident = consts.tile([128, 128], f32, tag="ident")
        make_identity(nc, ident)
        eps_t = consts.tile([128, 1], f32, tag="eps")
        nc.vector.memset(eps_t, 1e-12)

        kr = consts.tile([P, D], f32, tag="kr")
        nc.sync.dma_start(out=kr, in_=kr_d)
        k2 = consts.tile([P, D], f32, tag="k2")
        nc.vector.tensor_mul(k2, kr, kr)

        # k2T[d, dc, p]: transpose k2 through the PE
        k2T = consts.tile([128, DC, P], f32, tag="k2T")
        for dcc in range(DC):
            pst = ps_tr.tile([128, 512], f32, tag="tr")
            nc.tensor.transpose(
                pst[:, :P], k2[:, dcc * 128 : (dcc + 1) * 128], ident[:P, :P]
            )
            nc.scalar.copy(out=k2T[:, dcc, :], in_=pst[:, :P])

        # ---- s2 pipeline: load, transpose, square, rinv2 ----------------
        # s2T layout: [128(d), dc, b, m] fp32 (feeds rhsK build)
        s2T = big.tile([128, DC, BLOC, L], f32, tag="s2T")
        s2Tsq = big.tile([128, DC, BLOC, L], f32, tag="s2Tsq")
        for b in range(BLOC):
            natt2 = {}
            for lc in range(LC):
                natt = nat.tile([128, D], f32, tag="nat")
                nc.sync.dma_start(out=natt, in_=s2_d[b, lc * 128 : (lc + 1) * 128, :])
                natt2[lc] = natt
            # psum slab [128, 4*128] laid out (dc, lc) to match s2T free order
            slab = ps_tr.tile([128, 512], f32, tag="tr")
            for dcc in range(DC):
                for lc in range(LC):
                    o = (dcc * 2 + lc) * 128
                    nc.tensor.transpose(
                        slab[:, o : o + 128],
                        natt2[lc][:, dcc * 128 : (dcc + 1) * 128],
                        ident,
                    )
            slab3 = slab.rearrange("a (b c) -> a b c", b=DC)
            nc.scalar.copy(out=s2T[:, :, b, :], in_=slab3)
            nc.scalar.activation(out=s2Tsq[:, :, b, :], in_=slab3, func=Act.Square)

        # rinv2T[p, (b,m)] -> fp16, bounce through DRAM for the broadcast
        n2 = ps_nrm.tile([128, 512], f32, tag="nrm")
        for dcc in range(DC):
            nc.tensor.matmul(
                n2[:P, :],
                k2T[:, dcc, :],
                s2Tsq[:, dcc, :, :].rearrange("a b c -> a (b c)"),
                start=(dcc == 0),
                stop=(dcc == DC - 1),
            )
        sq2 = small.tile([P, BLOC * L], f32, tag="sq2")
        nc.scalar.activation(
            out=sq2, in_=n2[:P, :], func=Act.Sqrt, bias=eps_t[:P], scale=1.0
        )
        rinv2T = small.tile([P, BLOC * L], f16, tag="rinv2T")
        with nc.allow_low_precision(reason="rinv2 broadcast tile in fp16 is ample"):
            nc.vector.reciprocal(out=rinv2T, in_=sq2)
        r2d = dramp.tile([P, BLOC, L], f16, tag="r2d")
        nc.sync.dma_start(out=r2d, in_=rinv2T.rearrange("a (b c) -> a b c", b=BLOC))

        # broadcast to r2b[128, b, p, m] fp16 via partition-step-0 DRAM reads,
        # in chunks of 10 perspectives so the epilogue can start early
        r2b = big.tile([128, BLOC, P, L], f16, tag="r2b")
        r2d_flat = r2d.rearrange("a b c -> (a b c)")
        PCH = 10
        for b in range(BLOC):
            for ps0 in range(0, P, PCH):
                chunk = bass.AP(
                    tensor=r2d_flat.tensor,
                    offset=r2d_flat.offset + ps0 * BLOC * L + b * L,
                    ap=[[0, 128], [BLOC * L, PCH], [1, L]],
                )
                nc.sync.dma_start(out=r2b[:, b, ps0 : ps0 + PCH, :], in_=chunk)

        # ---- s1 pipeline: load, transpose (fp16 + squares), rinv1 -------
        s1T = big.tile([128, DC, BLOC, L], f16, tag="s1T")
        s1Tsq = big.tile([128, DC, BLOC, L], f32, tag="s1Tsq")
        for b in range(BLOC):
            natt1 = {}
            for lc in range(LC):
                natt = nat.tile([128, D], f32, tag="nat")
                nc.sync.dma_start(out=natt, in_=s1_d[b, lc * 128 : (lc + 1) * 128, :])
                natt1[lc] = natt
            slab = ps_tr.tile([128, 512], f32, tag="tr")
            for dcc in range(DC):
                for lc in range(LC):
                    o = (dcc * 2 + lc) * 128
                    nc.tensor.transpose(
                        slab[:, o : o + 128],
                        natt1[lc][:, dcc * 128 : (dcc + 1) * 128],
                        ident,
                    )
            slab3 = slab.rearrange("a (b c) -> a b c", b=DC)
            nc.scalar.copy(out=s1T[:, :, b, :], in_=slab3)
            nc.scalar.activation(out=s1Tsq[:, :, b, :], in_=slab3, func=Act.Square)

        rinv1 = {}
        for b in range(BLOC):
            for lc in range(LC):
                n1 = ps_nrm.tile([128, 512], f32, tag="nrm")
                for dcc in range(DC):
                    nc.tensor.matmul(
                        n1[:, :P],
                        s1Tsq[:, dcc, b, lc * 128 : (lc + 1) * 128],
                        k2T[:, dcc, :],
                        start=(dcc == 0),
                        stop=(dcc == DC - 1),
                    )
                sq1 = small.tile([128, P], f32, tag="sq1")
                nc.scalar.activation(
                    out=sq1, in_=n1[:, :P], func=Act.Sqrt, bias=eps_t, scale=1.0
                )
                r1 = small.tile([128, P], f32, tag="rinv1")
                nc.vector.reciprocal(out=r1, in_=sq1)
                rinv1[b, lc] = r1

        # ---- rhsK build: k2-scaled s2T in fp16 --------------------------
        # rhsK[d, dc, b, p, m]; pairs 0-4 on DVE (tensor_scalar runs 2x-mode
        # on fp32 SBUF, ~330ns/op, lands before the MULMAX stream), 5-9 on
        # ScalarE (catches up while the main loop runs)
        rhsK = big.tile([128, DC, BLOC, P, L], f16, tag="rhsK")
        for pair in range(NPAIR):
            for dcc in range(DC):
                for p in (2 * pair, 2 * pair + 1):
                    dst = rhsK[:, dcc, :, p, :]
                    src = s2T[:, dcc, :, :]
                    if pair >= 5:
                        nc.scalar.activation(
                            out=dst,
                            in_=src,
                            func=Act.Copy,
                            scale=k2T[:, dcc, p : p + 1],
                        )
                    else:
                        nc.vector.tensor_scalar_mul(dst, src, k2T[:, dcc, p : p + 1])

        # ---- main loop --------------------------------------------------
        # slabs of 5 pairs; within a slab all dc0 matmuls run back-to-back
        # with the same stationary operand, then all dc1 (no weight ping-pong)
        for b in range(BLOC):
            for lc in range(LC):
                maxt = outp.tile([128, P], f32, tag="maxt")
                for slab in range(0, NPAIR, 5):
                    pszs = [
                        ps_z.tile([128, 512], f32, tag="z", name=f"psz{j}")
                        for j in range(5)
                    ]
                    for dcc in range(DC):
                        for j in range(5):
                            pair = slab + j
                            nc.tensor.matmul(
                                pszs[j],
                                s1T[:, dcc, b, lc * 128 : (lc + 1) * 128],
                                rhsK[:, dcc, b, 2 * pair : 2 * pair + 2, :].rearrange(
                                    "a b c -> a (b c)"
                                ),
                                start=(dcc == 0),
                                stop=(dcc == DC - 1),
                            )
                    for j in range(5):
                        pair = slab + j
                        for i in range(2):
                            p = 2 * pair + i
                            nc.vector._custom_dve(
                                mulmax_op,
                                out=pszs[j][:, i * L : (i + 1) * L],
                                in0=pszs[j][:, i * L : (i + 1) * L],
                                in1=r2b[:, b, p, :],
                                accum_out=maxt[:, p : p + 1],
                            )
                outt = outp.tile([128, P], f32, tag="outt")
                nc.vector.tensor_mul(outt, maxt, rinv1[b, lc])
                nc.sync.dma_start(out=out_d[b, lc * 128 : (lc + 1) * 128, :], in_=outt)

    nc.compile()
    return nc


def _get_nc():
    if "nc" not in _NC_CACHE:
        _NC_CACHE["nc"] = build_nc()
    return _NC_CACHE["nc"]


def run(inputs, trace=False, trace_kwargs=None):
    from concourse.bass_utils import run_bass_kernel_spmd

    nc = _get_nc()
    sent1 = np.ascontiguousarray(np.asarray(inputs["sent1"], dtype=np.float32))
    sent2 = np.ascontiguousarray(np.asarray(inputs["sent2"], dtype=np.float32))
    kr = np.ascontiguousarray(np.asarray(inputs["kernel"], dtype=np.float32))
    in_maps = [
        {
            "sent1": sent1[i * BLOC : (i + 1) * BLOC],
            "sent2": sent2[i * BLOC : (i + 1) * BLOC],
            "kernel": kr,
        }
        for i in range(NCORES)
    ]
    res = run_bass_kernel_spmd(
        nc,
        in_maps,
        core_ids=list(range(NCORES)),
        trace=trace,
        **(trace_kwargs or {}),
    )
    out = np.concatenate([res.results[i]["out"] for i in range(NCORES)], axis=0)
    return out, res


def kernel(sent1, sent2, kernel):
    out, _ = run({"sent1": sent1, "sent2": sent2, "kernel": kernel})
    return out
